# revision 6
# baseline (speedup 1.0000x reference)
"""DLRM forward (nn_DLRM_Net_60301340835920) on 8 TRN2 NeuronCores.

Strategy: pure data parallelism over the batch (B=32768 -> 4096/core),
embedding tables replicated to every core (1.33GB f32 -> bf16 on host).
No collectives. Everything on-chip is feature-major [feat(part), batch(free)].

Per-core pipeline:
  bottom MLP (13->512->256->128, relu)            TensorE + ACT
  26 x indirect-DMA row gathers (bf16 tables)     SWDGE/SDMA
  PE-transpose gathered rows to [d, b]            TensorE
  pairwise interaction:  4-sample packed gram     TensorE
    matmuls [128,(r,i)=128] x [128,32] -> psum_r, 32-aligned block
    extraction -> transpose -> S4 [(i_lo,j), (i_hi, b)] layout
  top MLP fused per 512-batch chunk:
    FC1 = Wx @ xhat + sum_ih W4[ih] @ S4[.,ih,.]  (K=128 matmuls)
    FC2 (1024x1024), FC3 (1024->1) + sigmoid      TensorE + ACT

The padded feature axis is 32 slots (27 real: slot 0 = bottom-MLP output,
1..26 = embeddings, 27..31 = zeros). Top-layer-0 weights are expanded on the
host into the padded full-gram basis with 0.5 factors off-diagonal.
"""
import sys

sys.path.insert(0, "/opt/trn_rl_repo")

import numpy as np
import ml_dtypes

import concourse.bass as bass
from concourse import bacc
import concourse.mybir as mybir
import concourse.tile as tile
from concourse.masks import make_identity

P = 128
B = 32768
NCORES = 8
BL = B // NCORES          # 4096 per core
NT = BL // P              # 32 tiles per core
NTAB = 26
NI = 32                   # padded feature slots
NROWS = 100000
M = 128                   # embedding dim
CHUNK = 8                 # tiles per phase-A macro chunk
NCHUNK = NT // CHUNK      # 4

BF16 = mybir.dt.bfloat16
F32 = mybir.dt.float32
I32 = mybir.dt.int32
AF = mybir.ActivationFunctionType


def build_nc():
    nc = bacc.Bacc("TRN2", target_bir_lowering=False, debug=False)

    # ---- DRAM parameters -------------------------------------------------
    x0t = nc.declare_dram_parameter("x0t", [13, BL], BF16, isOutput=False)
    idx_d = nc.declare_dram_parameter("idx", [P, NTAB, NT], I32, isOutput=False)
    embs = [
        nc.declare_dram_parameter(f"emb{t}", [NROWS, M], BF16, isOutput=False)
        for t in range(NTAB)
    ]
    bw0t = nc.declare_dram_parameter("bw0t", [13, 512], BF16, isOutput=False)
    bw1t = nc.declare_dram_parameter("bw1t", [P, 4, 256], BF16, isOutput=False)
    bw2t = nc.declare_dram_parameter("bw2t", [P, 2, 128], BF16, isOutput=False)
    bb0_d = nc.declare_dram_parameter("bb0", [P, 4], F32, isOutput=False)
    bb1_d = nc.declare_dram_parameter("bb1", [P, 2], F32, isOutput=False)
    bb2_d = nc.declare_dram_parameter("bb2", [P, 1], F32, isOutput=False)
    wx_d = nc.declare_dram_parameter("wx", [M, 1024], BF16, isOutput=False)
    w4_d = nc.declare_dram_parameter("w4", [P, 7, 1024], BF16, isOutput=False)
    w2t_d = nc.declare_dram_parameter("w2t", [P, 8, 1024], BF16, isOutput=False)
    w3t_d = nc.declare_dram_parameter("w3t", [P, 8, 1], BF16, isOutput=False)
    tb0_d = nc.declare_dram_parameter("tb0", [P, 8], F32, isOutput=False)
    tb1_d = nc.declare_dram_parameter("tb1", [P, 8], F32, isOutput=False)
    tb2_d = nc.declare_dram_parameter("tb2", [1, 1], F32, isOutput=False)
    out_d = nc.declare_dram_parameter("out", [1, BL], F32, isOutput=True)

    with tile.TileContext(nc) as tc:
        with tc.tile_pool(name="pp", bufs=1) as pp:
            ident = pp.tile([P, P], BF16)
            make_identity(nc, ident[:])
            xhat = pp.tile([P, BL], BF16)
            out_sb = pp.tile([1, BL], F32)
            bb0 = pp.tile([P, 4], F32)
            bb1 = pp.tile([P, 2], F32)
            bb2 = pp.tile([P, 1], F32)
            tb0 = pp.tile([P, 8], F32)
            tb1 = pp.tile([P, 8], F32)
            tb2 = pp.tile([1, 1], F32)
            for d, s in ((bb0_d, bb0), (bb1_d, bb1), (bb2_d, bb2),
                         (tb0_d, tb0), (tb1_d, tb1), (tb2_d, tb2)):
                nc.sync.dma_start(out=s[:], in_=d[:])

            with tc.tile_pool(name="s4p", bufs=1) as s4p:
                # s4 [128=(i_lo,j), (i_hi, tau, beta)]  (i_hi 0..6; i=4*ih+il<=26)
                s4 = s4p.tile([P, 7, NT, P], BF16)

                # ============ bottom MLP ============
                with (
                    tc.tile_pool(name="botp", bufs=1) as botp,
                    tc.tile_pool(name="botps", bufs=4, space="PSUM") as botps,
                ):
                    x0 = botp.tile([13, BL], BF16)
                    nc.sync.dma_start(out=x0[:], in_=x0t[:])
                    w0 = botp.tile([13, 512], BF16)
                    nc.sync.dma_start(out=w0[:], in_=bw0t[:])
                    w1 = botp.tile([P, 4, 256], BF16)
                    nc.sync.dma_start(out=w1[:], in_=bw1t[:])
                    w2 = botp.tile([P, 2, 128], BF16)
                    nc.sync.dma_start(out=w2[:], in_=bw2t[:])
                    h1 = botp.tile([P, 4, BL], BF16)
                    h2 = botp.tile([P, 2, BL], BF16)
                    for m in range(4):
                        for bc in range(8):
                            ps = botps.tile([P, 512], F32, tag="bps", name="ps")
                            nc.tensor.matmul(
                                ps[:], w0[:, m * P:(m + 1) * P],
                                x0[:, bc * 512:(bc + 1) * 512],
                                start=True, stop=True)
                            nc.scalar.activation(
                                h1[:, m, bc * 512:(bc + 1) * 512], ps[:],
                                AF.Relu, bias=bb0[:, m:m + 1])
                    for m in range(2):
                        for bc in range(8):
                            ps = botps.tile([P, 512], F32, tag="bps", name="ps")
                            for k in range(4):
                                nc.tensor.matmul(
                                    ps[:],
                                    w1[:, k, m * P:(m + 1) * P],
                                    h1[:, k, bc * 512:(bc + 1) * 512],
                                    start=(k == 0), stop=(k == 3))
                            nc.scalar.activation(
                                h2[:, m, bc * 512:(bc + 1) * 512], ps[:],
                                AF.Relu, bias=bb1[:, m:m + 1])
                    for bc in range(8):
                        ps = botps.tile([P, 512], F32, tag="bps", name="ps")
                        for k in range(2):
                            nc.tensor.matmul(
                                ps[:], w2[:, k, :],
                                h2[:, k, bc * 512:(bc + 1) * 512],
                                start=(k == 0), stop=(k == 1))
                        nc.scalar.activation(
                            xhat[:, bc * 512:(bc + 1) * 512], ps[:],
                            AF.Relu, bias=bb2[:, 0:1])

                # ============ phase A: gather + transpose + gram ============
                with (
                    tc.tile_pool(name="phap", bufs=1) as phap,
                    tc.tile_pool(name="erp", bufs=2) as erp,
                    tc.tile_pool(name="g2p", bufs=2) as g2p,
                    tc.tile_pool(name="psrp", bufs=1, space="PSUM") as psrp,
                    tc.tile_pool(name="pstp", bufs=2, space="PSUM") as pstp,
                    tc.tile_pool(name="pebp", bufs=2, space="PSUM") as pebp,
                ):
                    idx = phap.tile([P, NTAB, NT], I32)
                    nc.sync.dma_start(out=idx[:], in_=idx_d[:])

                    for ck in range(NCHUNK):
                        ats = []
                        at3s = []
                        for tt in range(CHUNK):
                            at = phap.tile([P, P * NI], BF16,
                                           tag=f"at{tt}", name=f"at{tt}")
                            ats.append(at)
                            # layout: col(beta, i) at f = beta*NI + i
                            at3 = at[:].rearrange("d (b i) -> d b i", i=NI)
                            at3s.append(at3)
                            # zero pad slots, install xhat (strided dest)
                            nc.vector.memset(at3[:, :, 27:], 0.0)
                            nc.vector.tensor_copy(
                                out=at3[:, :, 0],
                                in_=xhat[:, (ck * CHUNK + tt) * P:
                                         (ck * CHUNK + tt + 1) * P])
                        # gathers + transposes, 4 tables at a time
                        for t0 in range(0, NTAB, 4):
                            tg = min(4, NTAB - t0)
                            ers = []
                            for dt in range(tg):
                                er = erp.tile([P, CHUNK, M], BF16,
                                              tag=f"er{dt}", name=f"er{dt}")
                                ers.append(er)
                                nc.gpsimd.indirect_dma_start(
                                    out=er[:],
                                    out_offset=None,
                                    in_=embs[t0 + dt][:],
                                    in_offset=bass.IndirectOffsetOnAxis(
                                        ap=idx[:, t0 + dt,
                                               ck * CHUNK:ck * CHUNK + CHUNK],
                                        axis=0),
                                )
                            for tt in range(CHUNK):
                                eb = pebp.tile([P, tg, M], BF16, tag="eb",
                                               name="eb")
                                for dt in range(tg):
                                    nc.tensor.transpose(
                                        eb[:, dt, :], ers[dt][:, tt, :],
                                        ident[:])
                                src = eb[:].rearrange("d t b -> d b t")
                                dst = at3s[tt][:, :, 1 + t0:1 + t0 + tg]
                                if tt % 2 == 0:
                                    nc.vector.tensor_copy(out=dst, in_=src)
                                else:
                                    nc.scalar.copy(out=dst, in_=src)
                        # gram + extraction per tile
                        for tt in range(CHUNK):
                            tau = ck * CHUNK + tt
                            g2 = g2p.tile([P, 32 * 32], BF16, tag="g2",
                                          name="g2")
                            at = ats[tt]
                            for c0 in (0, 16):
                                ps_r = [psrp.tile([P, 512], F32, tag=f"psr{r}",
                                                  name=f"psr{r}")
                                        for r in range(4)]
                                for g in range(c0, c0 + 16):
                                    lhsT = at[:, g * P:(g + 1) * P]
                                    for r in range(4):
                                        nc.tensor.matmul(
                                            ps_r[r][:, 32 * (g - c0):
                                                    32 * (g - c0 + 1)],
                                            lhsT,
                                            at[:, (4 * g + r) * NI:
                                               (4 * g + r + 1) * NI],
                                            start=True, stop=True)
                                for r in range(4):
                                    src = ps_r[r][32 * r:32 * (r + 1), :]
                                    dst = g2[32 * r:32 * (r + 1),
                                             32 * c0:32 * (c0 + 16)]
                                    if r % 2 == 0:
                                        nc.vector.tensor_copy(out=dst, in_=src)
                                    else:
                                        nc.scalar.copy(out=dst, in_=src)
                            # s4[:, ih, tau, beta]; beta = 4*g + r
                            s4v = s4[:, :, tau, :].rearrange(
                                "p ih (g r) -> p g r ih", r=4)
                            for q0 in range(0, 32, 8):
                                psT = pstp.tile([32, 8 * P], BF16, tag="psT",
                                                name="psT")
                                for k in range(8):
                                    g = q0 + k
                                    nc.tensor.transpose(
                                        psT[:, k * P:(k + 1) * P],
                                        g2[:, 32 * g:32 * (g + 1)], ident[:])
                                psT4 = psT[:].rearrange(
                                    "j (g8 r i) -> j g8 r i", g8=8, r=4)
                                for i_lo in range(4):
                                    src = psT4[:, :, :, i_lo::4][:, :, :, :7]
                                    dst = s4v[32 * i_lo:32 * (i_lo + 1),
                                              q0:q0 + 8, :, :]
                                    if i_lo % 2 == 0:
                                        nc.vector.tensor_copy(out=dst, in_=src)
                                    else:
                                        nc.scalar.copy(out=dst, in_=src)

                # ============ top MLP (fused FC1+FC2+FC3 per 512-chunk) ====
                with (
                    tc.tile_pool(name="fcp", bufs=1) as fcp,
                    tc.tile_pool(name="fch", bufs=2) as fch,
                    tc.tile_pool(name="fcps", bufs=4, space="PSUM") as fcps,
                    tc.tile_pool(name="fc3ps", bufs=2, space="PSUM") as fc3ps,
                ):
                    wx = fcp.tile([M, 1024], BF16)
                    nc.sync.dma_start(out=wx[:], in_=wx_d[:])
                    w4 = fcp.tile([P, 7, 1024], BF16)
                    nc.sync.dma_start(out=w4[:], in_=w4_d[:])
                    w2 = fcp.tile([P, 8, 1024], BF16)
                    nc.sync.dma_start(out=w2[:], in_=w2t_d[:])
                    w3 = fcp.tile([P, 8, 1], BF16)
                    nc.sync.dma_start(out=w3[:], in_=w3t_d[:])
                    for bc in range(8):
                        h3 = fch.tile([P, 8, 512], BF16, tag="h3", name="h3")
                        for n in range(8):
                            ps = fcps.tile([P, 512], F32, tag="fps", name="ps")
                            nc.tensor.matmul(
                                ps[:], wx[:, n * P:(n + 1) * P],
                                xhat[:, bc * 512:(bc + 1) * 512],
                                start=True, stop=False)
                            for ih in range(7):
                                rhs = s4[:, ih, 4 * bc:4 * bc + 4, :]
                                nc.tensor.matmul(
                                    ps[:], w4[:, ih, n * P:(n + 1) * P], rhs,
                                    start=False, stop=(ih == 6))
                            nc.scalar.activation(
                                h3[:, n, :], ps[:], AF.Relu,
                                bias=tb0[:, n:n + 1])
                        h4 = fch.tile([P, 8, 512], BF16, tag="h4", name="h4")
                        for m in range(8):
                            ps = fcps.tile([P, 512], F32, tag="fps", name="ps")
                            for k in range(8):
                                nc.tensor.matmul(
                                    ps[:], w2[:, k, m * P:(m + 1) * P],
                                    h3[:, k, :],
                                    start=(k == 0), stop=(k == 7))
                            nc.scalar.activation(
                                h4[:, m, :], ps[:], AF.Relu,
                                bias=tb1[:, m:m + 1])
                        ps3 = fc3ps.tile([1, 512], F32, tag="ps3", name="ps3")
                        for m in range(8):
                            nc.tensor.matmul(
                                ps3[:], w3[:, m, :],
                                h4[:, m, :],
                                start=(m == 0), stop=(m == 7))
                        nc.scalar.activation(
                            out_sb[:, bc * 512:(bc + 1) * 512], ps3[:],
                            AF.Sigmoid, bias=tb2[0:1, 0:1])
            nc.sync.dma_start(out=out_d[:], in_=out_sb[:])
    nc.compile()
    return nc


# ---------------------------------------------------------------------------
# host-side prep
# ---------------------------------------------------------------------------
_LI, _LJ = np.tril_indices(NTAB + 1, k=-1)
_TRI = {}
for p, (ii, jj) in enumerate(zip(_LI, _LJ)):
    _TRI[(ii, jj)] = p


def make_inputs(dense_x, lS_i, emb_W,
                bot_w0, bot_b0, bot_w1, bot_b1, bot_w2, bot_b2,
                top_w0, top_b0, top_w1, top_b1, top_w2, top_b2):
    """Build the per-core in_maps (list of dicts)."""
    bf = ml_dtypes.bfloat16
    # shared (replicated) tensors
    emb_bf = [np.ascontiguousarray(emb_W[t]).astype(bf) for t in range(NTAB)]
    shared = {f"emb{t}": emb_bf[t] for t in range(NTAB)}
    shared["bw0t"] = np.ascontiguousarray(bot_w0.T).astype(bf)       # [13,512]
    shared["bw1t"] = np.ascontiguousarray(
        bot_w1.T.reshape(4, P, 256).transpose(1, 0, 2)).astype(bf)
    shared["bw2t"] = np.ascontiguousarray(
        bot_w2.T.reshape(2, P, 128).transpose(1, 0, 2)).astype(bf)
    shared["bb0"] = np.ascontiguousarray(
        bot_b0.reshape(4, P).T).astype(np.float32)                   # [128,4]
    shared["bb1"] = np.ascontiguousarray(
        bot_b1.reshape(2, P).T).astype(np.float32)
    shared["bb2"] = np.ascontiguousarray(
        bot_b2.reshape(1, P).T).astype(np.float32)
    shared["wx"] = np.ascontiguousarray(top_w0[:, :M].T).astype(bf)  # [128,1024]
    # W4[ih, 32*i_lo + j, n] = W'[(i=4*ih+i_lo, j), n]
    wz = top_w0[:, M:]                                               # [1024, 351]
    w4 = np.zeros((P, 7, 1024), np.float32)
    for i in range(27):
        for j in range(27):
            if i == j:
                continue
            p = _TRI[(i, j)] if i > j else _TRI[(j, i)]
            ih, il = divmod(i, 4)
            w4[32 * il + j, ih, :] = 0.5 * wz[:, p]
    shared["w4"] = w4.astype(bf)
    shared["w2t"] = np.ascontiguousarray(
        top_w1.T.reshape(8, P, 1024).transpose(1, 0, 2)).astype(bf)
    shared["w3t"] = np.ascontiguousarray(
        top_w2.T.reshape(8, P, 1).transpose(1, 0, 2)).astype(bf)
    shared["tb0"] = np.ascontiguousarray(
        top_b0.reshape(8, P).T).astype(np.float32)
    shared["tb1"] = np.ascontiguousarray(
        top_b1.reshape(8, P).T).astype(np.float32)
    shared["tb2"] = np.asarray(top_b2, np.float32).reshape(1, 1)

    in_maps = []
    for c in range(NCORES):
        sl = slice(c * BL, (c + 1) * BL)
        m = dict(shared)
        m["x0t"] = np.ascontiguousarray(dense_x[sl].T).astype(bf)    # [13,4096]
        # idx[p, t, tau] = lS_i[t, c*BL + tau*128 + p]
        li = np.asarray(lS_i[:, sl], np.int32).reshape(NTAB, NT, P)  # [t,tau,p]
        m["idx"] = np.ascontiguousarray(li.transpose(2, 0, 1))       # [p,t,tau]
        in_maps.append(m)
    return in_maps


_NC_CACHE = {}


def kernel(dense_x, lS_i, lS_o, emb_W,
           bot_w0, bot_b0, bot_w1, bot_b1, bot_w2, bot_b2,
           top_w0, top_b0, top_w1, top_b1, top_w2, top_b2):
    del lS_o  # offsets are arange(B): pooling is identity
    from concourse.bass_utils import run_bass_kernel_spmd

    in_maps = make_inputs(dense_x, lS_i, emb_W,
                          bot_w0, bot_b0, bot_w1, bot_b1, bot_w2, bot_b2,
                          top_w0, top_b0, top_w1, top_b1, top_w2, top_b2)
    if "nc" not in _NC_CACHE:
        _NC_CACHE["nc"] = build_nc()
    res = run_bass_kernel_spmd(_NC_CACHE["nc"], in_maps,
                               core_ids=list(range(NCORES)))
    out = np.concatenate(
        [np.asarray(res.results[c]["out"], np.float32).reshape(BL)
         for c in range(NCORES)])
    return out.reshape(B, 1)
